# revision 1
# baseline (speedup 1.0000x reference)
"""Trainium2 Bass kernel for nn_Node2Property2 (segment_reduce).

Model: out = segment_sum(softplus_shifted(x @ W1 + b1) @ W2, batch, G)
  with softplus_shifted(v) = softplus(v) - log(2).

Strategy (8 NeuronCores, data-parallel over nodes):
  - Host pre-transposes x into xT [IN=128, N] layout and shards nodes
    contiguously across the 8 cores (replicated weights).
  - Device per core: stream xT tiles; hT = W1.T @ xT on the PE (float32r,
    full-rate); softplus via ScalarE Exp(bias=b1) then Ln(bias=1.0)
    (one table set: natural_log_exp_and_others); s = W2.T @ hT on the PE;
    per-node scalars DMA'd back out.
  - The sorted-segment combine runs on host in float64 (bincount), plus the
    fold of the -log(2) shift: P[g] -= count[g] * log2 * sum(W2).

kernel(**inputs) takes the FULL inputs and returns the FULL [G, 1] f32 output.
"""

import os
import sys

for _p in ("/opt/trn_rl_repo", "/root/.axon_site/_ro/trn_rl_repo"):
    if os.path.isdir(_p) and _p not in sys.path:
        sys.path.insert(0, _p)

import numpy as np

import concourse.bacc as bacc
import concourse.mybir as mybir
import concourse.tile as tile
from concourse.bass_utils import run_bass_kernel_spmd

F32 = mybir.dt.float32
F32R = mybir.dt.float32r
AF = mybir.ActivationFunctionType

LOG2 = float(np.log(2.0))

# Problem shape (fixed for this problem instance).
N, IN, H, OUT, G = 1048576, 128, 128, 1, 16384
NCORES = 8
NC_NODES = N // NCORES          # 131072 nodes per core

# Device tiling.
CH = 512                        # nodes per matmul chunk (f32 moving-dim max)
GRP = 8                         # chunks per group (= one DMA tile / Ln batch)
GRP_NODES = GRP * CH            # 4096
NGRP = NC_NODES // GRP_NODES    # 32 groups per core

# Pool buffer counts (overridable for tuning sweeps).
BUFS = {"xp": 3, "up": 2, "hp": 2, "stp": 2, "hps": 2, "sps": 2}


def _narrowed_act_tables(arch):
    """Narrow the act-table map so Exp and Ln are only offered by the set
    that contains BOTH (natural_log_exp_and_others). Otherwise the table-load
    placement alternates between exp_and_others and natural_log every group,
    paying a table reload each time. Entries keep their order, so the
    act_func_set_id indices stay aligned with act_info.json."""
    from concourse import hw_specs
    tables = hw_specs.get_activation_tables(arch)
    both = {AF.Exp, AF.Ln}
    keep = None
    for name, funcs in tables.items():
        if both <= funcs:
            keep = name
            break
    if keep is not None:
        for name, funcs in tables.items():
            if name != keep:
                funcs.difference_update(both)
    return tables


class _Bacc(bacc.Bacc):
    """Bacc with the narrowed act-table view for table-load placement."""

    def insert_act_table_loads(self):
        has_activation = any(
            isinstance(i, mybir.InstActivation)
            for b in self.main_func.blocks
            for i in b.instructions
        )
        if not has_activation:
            return
        tables = list(_narrowed_act_tables(self.m.arch).items())
        bacc._bass_rust.insert_act_table_loads(self, tables)


def _build_nc(repeat=1):
    nc = _Bacc("TRN2", target_bir_lowering=False, debug=False,
               num_devices=NCORES)
    xT = nc.declare_dram_parameter("xT", [IN, NC_NODES], F32R, isOutput=False)
    W1 = nc.declare_dram_parameter("W1", [IN, H], F32R, isOutput=False)
    b1 = nc.declare_dram_parameter("b1", [H, 1], F32, isOutput=False)
    W2 = nc.declare_dram_parameter("W2", [H, OUT], F32R, isOutput=False)
    s_out = nc.declare_dram_parameter("s", [NGRP, GRP_NODES], F32,
                                      isOutput=True)

    with tile.TileContext(nc) as tc:
        with (
            tc.tile_pool(name="wts", bufs=1) as wts,
            tc.tile_pool(name="xp", bufs=BUFS["xp"]) as xp,
            tc.tile_pool(name="up", bufs=BUFS["up"]) as up,
            tc.tile_pool(name="hp", bufs=BUFS["hp"]) as hp,
            tc.tile_pool(name="stp", bufs=BUFS["stp"]) as stp,
            tc.tile_pool(name="hps", bufs=BUFS["hps"], space="PSUM") as hps,
            tc.tile_pool(name="sps", bufs=BUFS["sps"], space="PSUM") as sps,
        ):
            w1r = wts.tile([IN, H], F32R)
            b1t = wts.tile([H, 1], F32)
            w2r = wts.tile([H, OUT], F32R)
            nc.sync.dma_start(w1r[:], W1[:])
            nc.sync.dma_start(b1t[:], b1[:])
            nc.sync.dma_start(w2r[:], W2[:])
            # Stage weights through DVE so each matmul waits on one producer.
            w1t = wts.tile([IN, H], F32R)
            nc.vector.tensor_copy(w1t[:], w1r[:])
            w2t = wts.tile([H, OUT], F32R)
            nc.vector.tensor_copy(w2t[:], w2r[:])

            def emit_mm2(g, h):
                """Second matmul + scalar collect + store for group g."""
                st = stp.tile([1, GRP_NODES], F32)
                for j2 in range(GRP // 2):
                    spt = sps.tile([1, 2 * CH], F32)
                    for k in range(2):
                        j = 2 * j2 + k
                        nc.tensor.matmul(
                            spt[0:1, k * CH:(k + 1) * CH], w2t[:],
                            h[:, j * CH:(j + 1) * CH],
                            start=True, stop=True)
                    nc.vector.tensor_copy(
                        st[0:1, j2 * 2 * CH:(j2 + 1) * 2 * CH], spt[:])
                nc.sync.dma_start(s_out[g:g + 1, :], st[:])

            # Software pipeline: group g's mm2 is emitted after group g+1's
            # mm1/exp, so the PE never sits behind a matmul that waits on the
            # ACT softplus chain of the current group.
            pending = None     # (g, h) awaiting mm2
            for g_rep in range(repeat * NGRP):
                g = g_rep % NGRP
                xt = xp.tile([IN, GRP_NODES], F32R)
                nc.sync.dma_start(
                    xt[:], xT[:, g * GRP_NODES:(g + 1) * GRP_NODES])

                u = up.tile([H, GRP_NODES], F32)
                for j in range(GRP // 2):
                    hpt = hps.tile([H, 2 * CH], F32)
                    for k in range(2):
                        c = 2 * j + k
                        nc.tensor.matmul(
                            hpt[:, k * CH:(k + 1) * CH],
                            w1t[:],
                            xt[:, c * CH:(c + 1) * CH],
                            start=True, stop=True,
                        )
                    # u = exp(v + b1), PSUM -> SBUF
                    nc.scalar.activation(
                        u[:, j * 2 * CH:(j + 1) * 2 * CH], hpt[:],
                        AF.Exp, bias=b1t[:], scale=1.0)

                if pending is not None:
                    emit_mm2(*pending)

                # h = ln(1 + u) = softplus(v + b1)
                h = hp.tile([H, GRP_NODES], F32R)
                nc.scalar.activation(h[:], u[:], AF.Ln, bias=1.0)
                pending = (g, h)

            emit_mm2(*pending)

    nc.compile()
    return nc


_NC_CACHE = {}


def _get_nc(repeat=1):
    if repeat not in _NC_CACHE:
        _NC_CACHE[repeat] = _build_nc(repeat)
    return _NC_CACHE[repeat]


def _run_device(x, W1, b1, W2, trace=False, tmpdir=None):
    """Returns per-node scalars s[n] = sum_k W2[k] * softplus((x@W1+b1)[n,k])
    (without the -log2 shift), plus the BassKernelResults."""
    nc = _get_nc()
    in_maps = []
    for i in range(NCORES):
        sl = slice(i * NC_NODES, (i + 1) * NC_NODES)
        xTi = np.ascontiguousarray(x[sl].T.astype(np.float32, copy=False))
        in_maps.append({
            "xT": xTi,
            "W1": np.ascontiguousarray(W1.astype(np.float32, copy=False)),
            "b1": np.ascontiguousarray(
                b1.astype(np.float32, copy=False).reshape(H, 1)),
            "W2": np.ascontiguousarray(
                W2.astype(np.float32, copy=False).reshape(H, OUT)),
        })
    res = run_bass_kernel_spmd(nc, in_maps, core_ids=list(range(NCORES)),
                               trace=trace, tmpdir=tmpdir)
    s_all = np.concatenate(
        [res.results[i]["s"].reshape(-1) for i in range(NCORES)])
    return s_all, res


def kernel(x, batch, W1, b1, W2, num_graphs):
    x = np.asarray(x)
    batch = np.asarray(batch)
    W1 = np.asarray(W1)
    b1 = np.asarray(b1)
    W2 = np.asarray(W2)
    g_count = int(num_graphs)
    assert x.shape == (N, IN) and batch.shape == (N,)

    s_all, _ = _run_device(x, W1, b1, W2)

    # Sorted-segment combine (host, f64), folding the -log(2) shift:
    # ref per-node value = s_n - log2 * sum(W2).
    idx = batch.astype(np.int64, copy=False)
    sums = np.bincount(idx, weights=s_all.astype(np.float64),
                       minlength=g_count)[:g_count]
    counts = np.bincount(idx, minlength=g_count)[:g_count]
    w2sum = float(np.asarray(W2, dtype=np.float64).sum())
    out = sums - counts * (LOG2 * w2sum)
    return out.astype(np.float32).reshape(g_count, OUT)

